# revision 41
# baseline (speedup 1.0000x reference)
"""Trainium2 Bass kernel for nn_CNNToLSTMCustomInterleaving.

Structure:
  launch 1 (8 cores, batch-sharded 2 rows/core):
    embedding gather -> PE-transpose -> 5 convs -> strided scatter
    eviction into re/im feature planes (bf16) -> feature sums, m4 partials,
    DMA-transpose -> bf16 Gram partials.
  host: sum Grams, covariance in true channel order (f64), eigh, top-300,
    fold Wih@top^T into Wtil, bias folding, fp8 repack of feat/weights.
  launch 2 (8 cores, time-split): fp8 DoubleRow pre + recurrent matmuls,
    tanh-identity gates (1 Act op per stream/step), fused scalar_tensor_tensor
    elementwise, fp8 hidden state; per-step hidden means.
  host: fuse means + MLP head -> [16] f32.
"""
import numpy as np
import ml_dtypes

import concourse.bass as bass
import concourse.bacc as bacc
import concourse.tile as tile
from concourse import mybir
from concourse.bass_utils import run_bass_kernel_spmd

fp32, bf16, i32 = mybir.dt.float32, mybir.dt.bfloat16, mybir.dt.int32
f8 = mybir.dt.float8e4
AF = mybir.ActivationFunctionType
OP = mybir.AluOpType
DR = mybir.MatmulPerfMode.DoubleRow
bfnp = ml_dtypes.bfloat16
f8np = ml_dtypes.float8_e4m3fn

B, T, E, H, V = 16, 4096, 300, 300, 130000
NPC = 300
EP = 320              # padded embedding/ci dim (launch 1)
CIB = [128, 128, 64]  # ci k-tiles (300 real + 20 zero)
COB = [128, 128, 48]  # conv out-channel tiles (44 real + 4 zero in last)
HP = 384              # per-gate padded to 3*128
G4 = 4 * HP           # 1536
# gate slot order in padded layout: slots [i, f, o, g] <- torch gates (i,f,g,o)
GSRC = [0, 1, 3, 2]   # torch gate index for padded slot k

_cache = {}


# --------------------------------------------------------------- launch 2
W2 = 32               # warm-up steps
NWIN = 24             # time windows over T (3 per core, overlapping by W2)
SW = 203              # steps per window (ceil(4096/24) + 32)
CH2 = 29              # steps per hw-loop iteration
NIT2 = SW // CH2      # 7
SOFF = [0, 96, 192]   # per-stream col offset within a window-step's 256 cols
# DoubleRow k-groups: (partition count) per group; each group covers 2*p chans
KGP = {0: [128, 128, 48], 1: [128, 128, 48], 2: [128, 24]}
WBLK = [0, 3, 6]      # wtil block index of each stream's kg0
NWB = 8               # total wtil (s,kg) blocks
HBP = [128, 44]       # rec k-group partition counts (k=256 pair + 44/two0)
STEPC = 3 * 256       # feat cols per step (3 windows x 256)
CHC = CH2 * STEPC     # feat cols per chunk
NFILL, NFILL2, FILLF = 6, 3, 256  # PE p-state fillers per round, free cols


def build_launch2():
    if "nc2" in _cache:
        return _cache["nc2"]
    nc = bacc.Bacc("TRN2", target_bir_lowering=False, debug=False, num_devices=8)
    feat_in = nc.dram_tensor("feat", [128, NIT2 * CHC], f8, kind="ExternalInput")
    wtil_in = nc.dram_tensor("wtil", [128, NWB * 2 * G4], f8, kind="ExternalInput")
    whh_in = nc.dram_tensor("whh", [128, 6 * 2 * G4], f8, kind="ExternalInput")
    scl_in = nc.dram_tensor("scl", [128, 4], fp32, kind="ExternalInput")
    m_out = nc.dram_tensor("m", [16, 9 * SW], fp32, kind="ExternalOutput")

    with tile.TileContext(nc) as tc:
        with tc.tile_pool(name="pp", bufs=1) as pp, \
             tc.tile_pool(name="fp", bufs=2) as fpp, \
             tc.tile_pool(name="sg", bufs=2) as sgp, \
             tc.tile_pool(name="zp", bufs=1, space="PSUM") as zpp, \
             tc.tile_pool(name="fl", bufs=1, space="PSUM") as flp, \
             tc.tile_pool(name="pm", bufs=1, space="PSUM") as pmp:
            wtil = pp.tile([128, NWB * 2 * G4], f8, tag="wtil", name="wtil")
            nc.sync.dma_start(wtil[:], wtil_in[:])
            whh = pp.tile([128, 6 * 2 * G4], f8, tag="whh", name="whh")
            nc.sync.dma_start(whh[:], whh_in[:])
            scl = pp.tile([128, 4], fp32, tag="scl", name="scl")
            nc.sync.dma_start(scl[:], scl_in[:])
            ones = pp.tile([128, 1], f8, tag="ones", name="ones")
            nc.vector.memset(ones[:], 1.0)
            means = pp.tile([16, 9 * SW], fp32, tag="means", name="means")
            # per window: H = 2h fp8 [128, 3s x 64] (cols 48:64 of each 64-block
            # are a zero pad for the rec kg1 DoubleRow pair); C'' = 2c fp32.
            hs = [pp.tile([128, 192], f8, tag=f"h{w}", name=f"h{w}") for w in range(3)]
            cs = [pp.tile([128, 144], fp32, tag=f"c{w}", name=f"c{w}") for w in range(3)]
            for t_ in hs:
                nc.vector.memset(t_[:], 0.0)
            for t_ in cs:
                nc.vector.memset(t_[:], 0.0)
            wv = wtil[:].rearrange("p (g two c) -> p g two c", g=NWB, two=2)
            hv = whh[:].rearrange("p (g two c) -> p g two c", g=6, two=2)
            filler = flp.tile([128, FILLF], fp32, tag="filler", name="filler")
            frhs = wtil[:, 0:2 * FILLF].rearrange("p (two b) -> p two b", two=2)

            def filler_mms(n):
                for _ in range(n):
                    nc.tensor.matmul(filler[:], wv[:, 0, :, 0:128], frhs,
                                     start=True, stop=True, perf_mode=DR)

            def pre_mms(w, z, fb, j):
                # opens the two psum zero-region groups of z (cols 0-511 and
                # 512-575); the matching stops are on the last rec matmuls.
                for s in range(3):
                    for kg in range(len(KGP[s])):
                        p = KGP[s][kg]
                        c0 = j * STEPC + 256 * w + SOFF[s] + 32 * kg
                        rhs = fb[:p, c0:c0 + 32].rearrange(
                            "p (two b) -> p two b", two=2)
                        for mt in range(12):
                            st = (kg == 0 and
                                  ((s == 0 and mt == 0) or (s == 2 and mt == 8)))
                            nc.tensor.matmul(
                                z[:, 192 * s + 16 * mt:192 * s + 16 * mt + 16],
                                wv[:p, WBLK[s] + kg, :, 128 * mt:128 * mt + 128],
                                rhs, start=st, stop=False, perf_mode=DR)

            def rec_mms(w, z):
                for s in range(3):
                    for kg in range(2):
                        p = HBP[kg]
                        rhs = hs[w][:p, 64 * s + 32 * kg:64 * s + 32 * kg + 32] \
                            .rearrange("p (two b) -> p two b", two=2)
                        for mt in range(12):
                            sp = (s == 2 and kg == 1 and mt in (7, 11))
                            nc.tensor.matmul(
                                z[:, 192 * s + 16 * mt:192 * s + 16 * mt + 16],
                                hv[:p, 2 * s + kg, :, 128 * mt:128 * mt + 128],
                                rhs, start=False, stop=sp, perf_mode=DR)

            with tc.For_i(0, NIT2, 1) as ic:
                fb = fpp.tile([128, CHC], f8, tag="fb", name="fb")
                nc.sync.dma_start(fb[:], feat_in[:, bass.ds(ic * CHC, CHC)])
                psm = pmp.tile([16, 9 * CH2], fp32, tag="pm", name="pm")
                zs = [zpp.tile([128, 1024], fp32, tag=f"z{w}", name=f"z{w}")
                      for w in range(3)]
                for w in range(3):
                    pre_mms(w, zs[w], fb, 0)
                    rec_mms(w, zs[w])
                def el_ops(w):
                    # s1 = (t_f+1)*C'' = 4fc; u1 = (t_i+1)*tg = 2*i*tg;
                    # C'' <- 0.5*s1 + u1 = 2c'   (all on DVE, one chain)
                    tv = ts[w][:].rearrange("p (s c) -> p s c", s=3)
                    s1s[w] = sgp.tile([128, 144], fp32, tag=f"s1{w}", name=f"s1{w}")
                    nc.vector.scalar_tensor_tensor(
                        s1s[w][:].rearrange("p (s c) -> p s c", s=3),
                        tv[:, :, 48:96], 1.0,
                        cs[w][:].rearrange("p (s c) -> p s c", s=3),
                        op0=OP.add, op1=OP.mult)
                    u1s[w] = sgp.tile([128, 144], bf16, tag=f"u1{w}", name=f"u1{w}")
                    nc.vector.scalar_tensor_tensor(
                        u1s[w][:].rearrange("p (s c) -> p s c", s=3),
                        tv[:, :, 0:48], 1.0, tv[:, :, 144:192],
                        op0=OP.add, op1=OP.mult)
                    nc.vector.scalar_tensor_tensor(
                        cs[w][:], s1s[w][:], 0.5, u1s[w][:],
                        op0=OP.mult, op1=OP.add)

                def gates(w):
                    ts[w] = sgp.tile([128, 576], bf16, tag=f"t{w}", name=f"t{w}")
                    nc.scalar.activation(ts[w][:], zs[w][:, 0:576], AF.Tanh,
                                         scale=scl[:, 0:1])

                def tanh_c(w):
                    tcs[w] = sgp.tile([128, 144], bf16, tag=f"tc{w}", name=f"tc{w}")
                    nc.scalar.activation(tcs[w][:], cs[w][:], AF.Tanh, scale=0.5)

                def h_op(w):
                    tv = ts[w][:].rearrange("p (s c) -> p s c", s=3)
                    hvw = hs[w][:].rearrange("p (s c) -> p s c", s=3)
                    nc.vector.scalar_tensor_tensor(
                        hvw[:, :, 0:48], tv[:, :, 96:144], 1.0,
                        tcs[w][:].rearrange("p (s c) -> p s c", s=3),
                        op0=OP.add, op1=OP.mult)

                def means_mms(j):
                    for w in range(3):
                        for s in range(3):
                            col = (3 * w + s) * CH2 + j
                            for kt in range(3):
                                p = [128, 128, 44][kt]
                                nc.tensor.matmul(
                                    psm[:, col:col + 1],
                                    hs[w][:p, 64 * s + 16 * kt:64 * s + 16 * kt + 16],
                                    ones[:p, :],
                                    start=(kt == 0), stop=(kt == 2))

                for j in range(CH2):
                    # fillers keep the PE busy (p-state ramp) across the
                    # dependency waits of this round's pre/rec matmuls
                    filler_mms(NFILL)
                    if j > 0:
                        means_mms(j - 1)   # one round late: H is long ready
                    ts, s1s, u1s, tcs = {}, {}, {}, {}
                    gates(0)
                    gates(1)
                    gates(2)
                    el_ops(0)
                    tanh_c(0)
                    el_ops(1)
                    h_op(0)
                    tanh_c(1)
                    el_ops(2)
                    h_op(1)
                    tanh_c(2)
                    h_op(2)
                    if j < CH2 - 1:
                        # step j+1 matmuls, per window, as soon as that
                        # window's z is released (pre) and H written (rec)
                        pre_mms(0, zs[0], fb, j + 1)
                        pre_mms(1, zs[1], fb, j + 1)
                        rec_mms(0, zs[0])
                        pre_mms(2, zs[2], fb, j + 1)
                        rec_mms(1, zs[1])
                        rec_mms(2, zs[2])
                    filler_mms(NFILL2)
                means_mms(CH2 - 1)
                # single fused evict per chunk; means is chunk-major
                # [16, (ic, ws, j)] and the host untangles the layout
                nc.scalar.activation(
                    means[:, bass.ds(ic * 9 * CH2, 9 * CH2)],
                    psm[:, 0:9 * CH2],
                    AF.Copy, scale=1.0 / 600.0)
            nc.sync.dma_start(m_out[:], means[:])
    nc.compile()
    _cache["nc2"] = nc
    return nc


def pack_launch2_weights(Wih, Whh, bih, bhh, top, mu, nch):
    """tanh-identity fold: returns (WP [1536, nch] f32 scaled-by-w_s with
    bias hi/lo in the last two channels, WH [1536, 304] f32, act_scale).
    top/mu in device layout (608/304 rows, pads zero)."""
    topc = top[:nch]
    Wt = (Wih.astype(np.float64) @ topc.astype(np.float64).T).astype(np.float64)
    btl = (bih + bhh).astype(np.float64) - Wt @ mu[:nch].astype(np.float64)
    Wt[600:900] *= 2.0   # g rows doubled (tanh identity)
    btl[600:900] *= 2.0
    Wh = Whh.astype(np.float64) * 0.5
    Wh[600:900] *= 2.0
    # scales: keep fp8 operands in a healthy range (whh/bias bound applied
    # by the caller, which knows the feat scale)
    w_s = 224.0 / max(np.abs(Wt).max(), 1e-6)
    WP = np.zeros((G4, nch), np.float64)
    WHP = np.zeros((G4, 304), np.float64)
    bfull = np.zeros(G4, np.float64)
    for k in range(4):
        g = GSRC[k]
        WP[HP * k:HP * k + 300, :] = Wt[300 * g:300 * g + 300, :]
        WHP[HP * k:HP * k + 300, :300] = Wh[300 * g:300 * g + 300, :]
        bfull[HP * k:HP * k + 300] = btl[300 * g:300 * g + 300]
    return WP, WHP, bfull, w_s


def host_pca_from_G(G, s_sum, imonly):
    """G, s_sum in device layout. up/mid: 608 (re304|im304 padded);
    low: 304 (im only). Returns mu, top f32 (device layout, pads zero)."""
    N = B * T
    if imonly:
        true_idx = 1 + 2 * np.arange(300)
        G_r = G[np.ix_(np.arange(300), np.arange(300))].astype(np.float64)
        mu_r = (s_sum[:300] / N).astype(np.float64)
        cov600 = np.zeros((600, 600))
        cov600[np.ix_(true_idx, true_idx)] = (G_r - N * np.outer(mu_r, mu_r)) / (B - 1)
    else:
        mine_real = np.concatenate([np.arange(300), 304 + np.arange(300)])
        true_idx = np.concatenate([2 * np.arange(300), 1 + 2 * np.arange(300)])
        G_r = G[np.ix_(mine_real, mine_real)].astype(np.float64)
        mu_r = (s_sum[mine_real] / N).astype(np.float64)
        cov600 = np.zeros((600, 600))
        cov600[np.ix_(true_idx, true_idx)] = (G_r - N * np.outer(mu_r, mu_r)) / (B - 1)
    evals, evecs = np.linalg.eigh(cov600)
    top_true = evecs[:, np.argsort(-evals)[:NPC]]   # [600, NPC]
    nchp = 304 if imonly else 608
    top608 = np.zeros((nchp, NPC), np.float32)
    mu608 = np.zeros(nchp, np.float32)
    if imonly:
        top608[:300] = top_true[true_idx].astype(np.float32)
        mu608[:300] = mu_r.astype(np.float32)
    else:
        top608[mine_real] = top_true[true_idx].astype(np.float32)
        mu608[mine_real] = mu_r.astype(np.float32)
    return mu608, top608


# --------------------------------------------------------------- launch 1
CONVS = {  # name: (K, stride, pad, tlo, Lout)
    "y2": (2, 1, 0, 0, 2047),
    "y4": (4, 2, 0, 0, 1023),
    "y3": (3, 3, 2, 1, 682),
    "y6": (6, 3, 2, 0, 683),
    "y5": (5, 3, 0, 0, 682),
}
CWORD = ["y2", "y4", "y3", "y6", "y5"]
CWCOLS = {}
_off = 0
for _nm in CWORD:
    for _tap in range(CONVS[_nm][0]):
        for _kt in range(3):
            CWCOLS[(_nm, _tap, _kt)] = _off
            _off += 304
CWTOT = _off  # 18240

# scatter: feat[fstr*(u-ubase)+off] = y[u] for u in [ulo, uhi)
SCAT = {
    "y2": (2, 0, (1, 2), 0, 2047),
    "y4": (4, 0, (1, 3, 4, 6), 0, 1023),
    "y3": (6, 0, (3, 5, 7), 0, 682),
    "y6": (6, 0, (-3, -1, 1, 2, 4, 6), 1, 682),
    "y5": (6, 1, (1, 3, 5, 6, 8), 1, 682),
}


def build_launch1():
    if "nc1" in _cache:
        return _cache["nc1"]
    nc = bacc.Bacc("TRN2", target_bir_lowering=False, debug=False, num_devices=8)
    x_in = nc.dram_tensor("xr", [2, 128, 32], i32, kind="ExternalInput")
    emb_in = nc.dram_tensor("emb", [V, EP], bf16, kind="ExternalInput")
    cw_in = nc.dram_tensor("cw", [128, CWTOT], bf16, kind="ExternalInput")
    cb_in = nc.dram_tensor("cb", [128, 15], fp32, kind="ExternalInput")
    fu_out = nc.dram_tensor("fu", [2, 6, 128, T], bf16, kind="ExternalOutput")
    fm_out = nc.dram_tensor("fm", [2, 6, 128, T], bf16, kind="ExternalOutput")
    fl_out = nc.dram_tensor("fl", [2, 3, 128, T], bf16, kind="ExternalOutput")
    gu_out = nc.dram_tensor("gu", [5, 128, 608], fp32, kind="ExternalOutput")
    gm_out = nc.dram_tensor("gm", [5, 128, 608], fp32, kind="ExternalOutput")
    gl_out = nc.dram_tensor("gl", [3, 128, 304], fp32, kind="ExternalOutput")

    STREAMS = [("u", ["y2", "y4"], 608, fu_out, gu_out),
               ("m", ["y3", "y6"], 608, fm_out, gm_out),
               ("l", ["y5"], 304, fl_out, gl_out)]
    SUMCOL = {"u": 0, "m": 12, "l": 24}

    from concourse.masks import make_identity
    with tile.TileContext(nc) as tc:
        with tile.ExitStack() if False else __import__("contextlib").ExitStack() as ctx:
            pp = ctx.enter_context(tc.tile_pool(name="pp", bufs=1))
            gat = ctx.enter_context(tc.tile_pool(name="gat", bufs=3))
            cwp = ctx.enter_context(tc.tile_pool(name="cwp", bufs=2))
            featp = ctx.enter_context(tc.tile_pool(name="featp", bufs=1))
            ftp = ctx.enter_context(tc.tile_pool(name="ftp", bufs=3))
            gaccp = ctx.enter_context(tc.tile_pool(name="gacc", bufs=1))
            ps_c = ctx.enter_context(tc.tile_pool(name="ps_c", bufs=2, space="PSUM"))
            ps_g = ctx.enter_context(tc.tile_pool(name="ps_g", bufs=1, space="PSUM"))

            identf = pp.tile([128, 128], fp32, tag="identf", name="identf")
            make_identity(nc, identf[:])
            ident = pp.tile([128, 128], bf16, tag="ident", name="ident")
            nc.vector.tensor_copy(ident[:], identf[:])
            xidx = pp.tile([128, 64], i32, tag="xidx", name="xidx")
            nc.sync.dma_start(xidx[:, 0:32], x_in[0])
            nc.sync.dma_start(xidx[:, 32:64], x_in[1])
            cbias = pp.tile([128, 15], fp32, tag="cbias", name="cbias")
            nc.sync.dma_start(cbias[:], cb_in[:])
            xc = [pp.tile([CIB[k], T], bf16, tag=f"xc{k}", name=f"xc{k}") for k in range(3)]
            gacc = {"u": gaccp.tile([128, 5 * 608], fp32, tag="gu", name="gu"),
                    "m": gaccp.tile([128, 5 * 608], fp32, tag="gm", name="gm"),
                    "l": gaccp.tile([128, 3 * 304], fp32, tag="gl", name="gl")}
            for g_ in gacc.values():
                nc.vector.memset(g_[:], 0.0)

            for r in range(2):
                for g in range(32):
                    xt = gat.tile([128, EP], bf16, tag="xt", name="xt")
                    nc.gpsimd.indirect_dma_start(
                        out=xt[:], out_offset=None, in_=emb_in[:],
                        in_offset=bass.IndirectOffsetOnAxis(
                            ap=xidx[:, 32 * r + g:32 * r + g + 1], axis=0))
                    for kt in range(3):
                        pt = ps_c.tile([128, 512], fp32, tag="psc", name="psc")
                        ptb = pt[:CIB[kt], 0:64].bitcast(bf16)
                        nc.tensor.transpose(
                            ptb, xt[:, 128 * kt:128 * kt + CIB[kt]], ident[:])
                        nc.vector.tensor_copy(xc[kt][:, g::32], ptb)

                # conv pass + gram pass per stream, interleaved so the gram
                # DMA-transposes of one stream overlap the next stream's conv
                # matmuls. l reuses u's feat tags (u's gram is done by then).
                all_fts = {}
                TAGP = {"u": "u", "m": "m", "l": "u"}

                def conv_pass(snm, convs, nch, f_out, g_out_t):
                    ntile = 6 if nch == 608 else 3
                    nmt = 5 if nch == 608 else 3
                    fts = [featp.tile([128, T], bf16, tag=f"{TAGP[snm]}ft{i}",
                                      name=f"{snm}ft{i}")
                           for i in range(ntile)]
                    all_fts[snm] = fts
                    for fi, ft in enumerate(fts):
                        if fi % 2 == 0:
                            nc.vector.memset(ft[:], 0.0)
                        else:
                            nc.gpsimd.memset(ft[:], 0.0)
                    for cnm in convs:
                        K, stride, pad, tlo, Lout = CONVS[cnm]
                        isim = (cnm in ("y4", "y6"))
                        base_t = 3 if isim else 0
                        cw_sb = cwp.tile([128, K * 3 * 304], bf16, tag="cw", name="cw")
                        nc.sync.dma_start(
                            cw_sb[:],
                            cw_in[:, CWCOLS[(cnm, 0, 0)]:CWCOLS[(cnm, 0, 0)] + K * 3 * 304])
                        cbcol = 3 * CWORD.index(cnm)
                        fstr, ubase, offs, ulo, uhi = SCAT[cnm]
                        c0 = 0
                        while c0 < Lout:
                            n = min(512, Lout - c0)
                            for mt in range(3):
                                psc = ps_c.tile([128, 512], fp32, tag="psc", name="psc")
                                nmm = [(tap, kt) for tap in range(K) for kt in range(3)]
                                full = [(tap, kt) for tap, kt in nmm
                                        if stride * (tlo + c0) + tap - pad >= 0]
                                part = [(tap, kt) for tap, kt in nmm
                                        if stride * (tlo + c0) + tap - pad < 0]
                                for idx2, (tap, kt) in enumerate(full + part):
                                    a = stride * (tlo + c0) + tap - pad
                                    wsl = cw_sb[:CIB[kt],
                                                (tap * 3 + kt) * 304 + 128 * mt:
                                                (tap * 3 + kt) * 304 + 128 * mt + COB[mt]]
                                    if a >= 0:
                                        nc.tensor.matmul(
                                            psc[:COB[mt], 0:n], wsl,
                                            xc[kt][:, a:a + stride * n:stride],
                                            start=(idx2 == 0),
                                            stop=(idx2 == len(nmm) - 1))
                                    else:
                                        nskip = ((-a + stride - 1) // stride)
                                        nc.tensor.matmul(
                                            psc[:COB[mt], nskip:n], wsl,
                                            xc[kt][:, a + stride * nskip:
                                                   a + stride * n:stride],
                                            start=False,
                                            stop=(idx2 == len(nmm) - 1))
                                lo = max(ulo, c0)
                                hi = min(uhi, c0 + n)
                                if hi > lo:
                                    cnt = hi - lo
                                    for off in offs:
                                        fc0 = fstr * (lo - ubase) + off
                                        nc.scalar.activation(
                                            fts[base_t + mt][:COB[mt],
                                                             fc0:fc0 + fstr * (cnt - 1) + 1:fstr],
                                            psc[:COB[mt], lo - c0:lo - c0 + cnt],
                                            AF.Identity,
                                            bias=cbias[:COB[mt], cbcol + mt:cbcol + mt + 1])
                                if cnm == "y6" and c0 == 0:
                                    for ec in (1, 2, 4, 6):
                                        nc.scalar.activation(
                                            fts[base_t + mt][:COB[mt], ec:ec + 1],
                                            psc[:COB[mt], 0:1], AF.Identity,
                                            bias=cbias[:COB[mt], cbcol + mt:cbcol + mt + 1])
                                if cnm == "y6" and c0 + n == Lout:
                                    for ec in (4089, 4091, 4093, 4094):
                                        nc.scalar.activation(
                                            fts[base_t + mt][:COB[mt], ec:ec + 1],
                                            psc[:COB[mt], n - 1:n], AF.Identity,
                                            bias=cbias[:COB[mt], cbcol + mt:cbcol + mt + 1])
                            c0 += n
                    # ---- feat out (sums are computed on the host); tail
                    # tiles only carry 48 used rows, so skip the pad rows
                    for i, ft in enumerate(fts):
                        nr = COB[i % 3]
                        nc.sync.dma_start(f_out[r, i, 0:nr], ft[:nr])

                def gram_pass(snm, convs, nch, f_out, g_out_t):
                    nmt = 5 if nch == 608 else 3
                    fts = all_fts[snm]
                    # ---- gram: DMA-transpose chunks + bf16 matmuls
                    nchp = 608 if nch == 608 else 304
                    gm_ps = [ps_g.tile([128, 512], fp32, tag=f"g{j}", name=f"g{j}")
                             for j in range(nmt)] if nch == 608 else \
                            [ps_g.tile([128, 304], fp32, tag=f"g{j}", name=f"g{j}")
                             for j in range(nmt)]
                    grem = ps_g.tile([128, 96], fp32, tag="grem", name="grem") \
                        if nch == 608 else None
                    for tch in range(4):
                        ftt = ftp.tile([128, 8 * nchp], bf16, tag="ftt", name="ftt")
                        ftt_v = ftt[:].rearrange("p (b c) -> p b c", c=nchp)
                        for i, ft in enumerate(fts):
                            pw = COB[i % 3]
                            ch0 = 304 * (i // 3) + 128 * (i % 3)
                            nc.sync.dma_start_transpose(
                                ftt_v[:, :, ch0:ch0 + pw],
                                ft[:pw, 1024 * tch:1024 * (tch + 1)])
                        for blk in range(8):
                            first = (tch == 0 and blk == 0)
                            last = (tch == 3 and blk == 7)
                            for j in range(nmt):
                                mw = 128 if 128 * (j + 1) <= nchp else nchp - 128 * j
                                lhs = ftt_v[:, blk, 128 * j:128 * j + mw]
                                # upper triangle only; host mirrors the rest
                                if nch == 608:
                                    if j == 0:
                                        nc.tensor.matmul(
                                            gm_ps[0][:mw, :], lhs,
                                            ftt_v[:, blk, 0:512],
                                            start=first, stop=last)
                                        nc.tensor.matmul(
                                            grem[:mw, 0:96], lhs,
                                            ftt_v[:, blk, 512:608],
                                            start=first, stop=last)
                                    else:
                                        wdt = 608 - 128 * j
                                        nc.tensor.matmul(
                                            gm_ps[j][:mw, :wdt], lhs,
                                            ftt_v[:, blk, 128 * j:608],
                                            start=first, stop=last)
                                else:
                                    wdt = 304 - 128 * j
                                    nc.tensor.matmul(
                                        gm_ps[j][:mw, :wdt], lhs,
                                        ftt_v[:, blk, 128 * j:304],
                                        start=first, stop=last)
                    # accumulate G (upper-triangle blocks) into sbuf
                    ga = gacc[snm]
                    for j in range(nmt):
                        mwj = min(128, (608 if nch == 608 else 304) - 128 * j)
                        if nch == 608:
                            if j == 0:
                                nc.vector.tensor_tensor(
                                    ga[:mwj, 0:512], ga[:mwj, 0:512],
                                    gm_ps[0][:mwj, :], op=OP.add)
                                nc.vector.tensor_tensor(
                                    ga[:mwj, 512:608], ga[:mwj, 512:608],
                                    grem[:mwj, 0:96], op=OP.add)
                            else:
                                wdt = 608 - 128 * j
                                c0 = 608 * j + 128 * j
                                nc.vector.tensor_tensor(
                                    ga[:mwj, c0:c0 + wdt],
                                    ga[:mwj, c0:c0 + wdt],
                                    gm_ps[j][:mwj, :wdt], op=OP.add)
                        else:
                            wdt = 304 - 128 * j
                            c0 = 304 * j + 128 * j
                            nc.vector.tensor_tensor(
                                ga[:mwj, c0:c0 + wdt],
                                ga[:mwj, c0:c0 + wdt],
                                gm_ps[j][:mwj, :wdt], op=OP.add)

                conv_pass(*STREAMS[0])
                conv_pass(*STREAMS[1])
                gram_pass(*STREAMS[0])
                conv_pass(*STREAMS[2])
                gram_pass(*STREAMS[1])
                gram_pass(*STREAMS[2])
            for snm, _c, nch, _f, g_out_t in STREAMS:
                nmt = 5 if nch == 608 else 3
                w = 608 if nch == 608 else 304
                for j in range(nmt):
                    nc.sync.dma_start(g_out_t[j], gacc[snm][:, w * j:w * (j + 1)])
    nc.compile()
    _cache["nc1"] = nc
    return nc


def pack_launch1_inputs(x_np, emb_np, inp):
    xr = np.zeros((2, 128, 32), np.int32)
    embp = np.zeros((V, EP), bfnp)
    embp[:, :300] = emb_np.astype(bfnp)
    cw = np.zeros((128, CWTOT), bfnp)
    for nm in CWORD:
        K = CONVS[nm][0]
        w = inp["w" + nm[1]]  # [300, 300, K]
        for tap in range(K):
            wt = w[:, :, tap]  # [co, ci]
            for kt in range(3):
                nci = CIB[kt] if kt < 2 else 44
                rows = wt[:, 128 * kt:128 * kt + nci].T  # [ci, co=300]
                c0 = CWCOLS[(nm, tap, kt)]
                cw[:nci, c0:c0 + 300] = rows
    cb = np.zeros((128, 15), np.float32)
    for qi, nm in enumerate(CWORD):
        b = inp["b" + nm[1]]
        for mt in range(3):
            nr = COB[mt] if mt < 2 else 44
            cb[:nr, 3 * qi + mt] = b[128 * mt:128 * mt + nr]
    return xr, embp, cw, cb


def _f8(x):
    return np.asarray(x, np.float32).astype(f8np)


# --------------------------------------------------------------- full kernel
def kernel(**inputs):
    inp = {k: np.asarray(v) for k, v in inputs.items()}
    x = inp["x"].astype(np.int64)

    # ---- launch 1
    nc1 = build_launch1()
    _, embp, cw, cb = pack_launch1_inputs(None, inp["emb"], inp)
    in_maps = []
    for c in range(8):
        xr = np.zeros((2, 128, 32), np.int32)
        for r in range(2):
            xr[r] = x[2 * c + r].reshape(128, 32).astype(np.int32)
        in_maps.append({"xr": xr, "emb": embp, "cw": cw, "cb": cb})
    res1 = run_bass_kernel_spmd(nc1, in_maps, core_ids=list(range(8)))

    # ---- host: gram totals, eigh, weight folding, feat repack
    G = {"u": np.zeros((608, 608), np.float64),
         "m": np.zeros((608, 608), np.float64),
         "l": np.zeros((304, 304), np.float64)}
    f608 = {s_: np.zeros((608, B, T), np.float32) for s_ in ("u", "m", "l")}
    for c in range(8):
        r1 = res1.results[c]
        for s_, gk, nmt, w in (("u", "gu", 5, 608), ("m", "gm", 5, 608),
                               ("l", "gl", 3, 304)):
            gdev = r1[gk]
            gfull = np.concatenate([gdev[j] for j in range(nmt)], 0)[:w].astype(np.float64)
            # device computed upper-triangle blocks only; mirror the rest
            for j in range(1, nmt):
                r0 = 128 * j
                r1e = min(w, r0 + 128)
                gfull[r0:r1e, 0:r0] = gfull[0:r0, r0:r1e].T
            G[s_] += gfull
            nt = 6 if w == 608 else 3
            fdev = r1["fu" if s_ == "u" else ("fm" if s_ == "m" else "fl")]
            for r in range(2):
                for i in range(nt):
                    pl, sub = divmod(i, 3)
                    nr = [128, 128, 48][sub]
                    ch0 = 304 * pl + 128 * sub
                    f608[s_][ch0:ch0 + nr, 2 * c + r] = fdev[r, i, :nr].astype(np.float32)
    # feature sums and the embedding mean on the host (exact inputs for m4)
    S = {s_: f608[s_].sum(axis=(1, 2), dtype=np.float64) for s_ in ("u", "m", "l")}
    S["l"] = S["l"][:304]
    m4 = inp["emb"].astype(np.float32)[x].mean(axis=-1)
    pca = {}
    for s_, imonly in (("u", False), ("m", False), ("l", True)):
        pca[s_] = host_pca_from_G(G[s_].astype(np.float32),
                                  S[s_].astype(np.float32), imonly)

    # ---- launch 2 packing: fp8 weights + feat
    nc2 = build_launch2()
    # global scale pair shared by all three streams (one Act scale per gates op)
    packs = {}
    wmax, fmax = 0.0, 0.0
    for s, (s_, nch) in enumerate((("u", 608), ("m", 608), ("l", 304))):
        mu608, top608 = pca[s_]
        WP, WHP, bfull, _ = pack_launch2_weights(
            inp[s_ + "Wih"], inp[s_ + "Whh"], inp[s_ + "bih"], inp[s_ + "bhh"],
            top608, mu608, nch)
        packs[s] = (WP, WHP, bfull, nch)
        wmax = max(wmax, np.abs(WP).max())
        fmax = max(fmax, np.abs(f608[s_][:nch]).max())
    # stay well under 240 (e4m3-with-inf max) so no packed byte can decode
    # as inf/nan regardless of which fp8 e4m3 flavor the hardware assumes
    s_g = 192.0 / max(fmax, 1e-6)
    w_g = 192.0 / max(wmax, 1e-6)
    lim = max(max(np.abs(p[1]).max() for p in packs.values()) * s_g * w_g,
              max(np.abs(p[2]).max() for p in packs.values()) * s_g * w_g)
    if lim > 208.0:
        w_g *= 208.0 / lim
    scl_np = np.zeros((128, 4), np.float32)
    scl_np[:, 0] = 0.5 / (s_g * w_g)

    wtil8 = np.zeros((128, NWB, 2, G4), f8np)
    whh8 = np.zeros((128, 6, 2, G4), f8np)
    fscaled = {}
    for s, (s_, nch) in enumerate((("u", 608), ("m", 608), ("l", 304))):
        WP, WHP, bfull, _ = packs[s]
        WPs = WP * w_g                          # [1536, nch]
        WHs = WHP * s_g * w_g                   # [1536, 304]
        bsc = bfull * s_g * w_g
        b_hi = _f8(bsc).astype(np.float64)
        b_lo = bsc - b_hi
        fscaled[s_] = f608[s_][:nch] * s_g
        nkg = len(KGP[s])
        for kg in range(nkg):
            p = KGP[s][kg]
            ch0 = 256 * kg
            for two in range(2):
                lo = ch0 + two * p
                hi = min(lo + p, nch)
                n = hi - lo
                if n <= 0:
                    continue
                wtil8[:n, WBLK[s] + kg, two, :] = _f8(WPs[:, lo:hi].T)
        # bias rows: channels nch-2 (hi), nch-1 (lo) live in the last kg
        kgb = nkg - 1
        pb = KGP[s][kgb]
        for ch, bv in ((nch - 2, b_hi), (nch - 1, b_lo)):
            two = (ch - 256 * kgb) // pb
            prow = (ch - 256 * kgb) % pb
            wtil8[prow, WBLK[s] + kgb, two, :] = _f8(bv)
        whh8[:, 2 * s + 0, 0, :] = _f8(WHs[:, 0:128].T)
        whh8[:, 2 * s + 0, 1, :] = _f8(WHs[:, 128:256].T)
        whh8[:44, 2 * s + 1, 0, :] = _f8(WHs[:, 256:300].T)

    WSPAN = [( (T * wi) // NWIN, (T * (wi + 1)) // NWIN ) for wi in range(NWIN)]
    in2 = []
    for c in range(8):
        fv = np.zeros((128, SW, STEPC), f8np)
        for w in range(3):
            wi = 3 * c + w
            lo_g = WSPAN[wi][0]
            t0 = 0 if wi == 0 else lo_g - W2
            tg = t0 + np.arange(SW)
            for s, (s_, nch) in enumerate((("u", 608), ("m", 608), ("l", 304))):
                fsc = fscaled[s_]
                for kg in range(len(KGP[s])):
                    p = KGP[s][kg]
                    ch0 = 256 * kg
                    for two in range(2):
                        lo = ch0 + two * p
                        hi = min(lo + p, nch)
                        n = hi - lo
                        if n <= 0:
                            continue
                        blk = fsc[lo:hi][:, :, tg]          # [n, B, SW]
                        col = 256 * w + SOFF[s] + 32 * kg + 16 * two
                        fv[:n, :, col:col + 16] = \
                            _f8(blk.transpose(0, 2, 1))     # [n, SW, 16]
                nkg = len(KGP[s])
                kgb = nkg - 1
                pb = KGP[s][kgb]
                for ch in (nch - 2, nch - 1):
                    two = (ch - 256 * kgb) // pb
                    prow = (ch - 256 * kgb) % pb
                    col = 256 * w + SOFF[s] + 32 * kgb + 16 * two
                    fv[prow, :, col:col + 16] = f8np(1.0)
        in2.append({"feat": fv.reshape(128, NIT2 * CHC),
                    "wtil": wtil8.reshape(128, NWB * 2 * G4),
                    "whh": whh8.reshape(128, 6 * 2 * G4), "scl": scl_np})
    res2 = run_bass_kernel_spmd(nc2, in2, core_ids=list(range(8)))
    ms = np.zeros((3, B, T), np.float32)
    for c in range(8):
        mraw = res2.results[c]["m"]  # [16, NIT2*9*CH2] chunk-major
        mc = mraw.reshape(16, NIT2, 9, CH2).transpose(0, 2, 1, 3) \
                 .reshape(16, 9 * SW)
        for w in range(3):
            wi = 3 * c + w
            lo_g, hi_g = WSPAN[wi]
            t0 = 0 if wi == 0 else lo_g - W2
            for s in range(3):
                blk = mc[:, (3 * w + s) * SW:(3 * w + s + 1) * SW]
                ms[s][:, lo_g:hi_g] = blk[:, lo_g - t0:hi_g - t0]
    m1, m2, m3 = ms[0], ms[1], ms[2]

    # ---- head (host, f32)
    fw = inp["fuse_w"].astype(np.float32)
    fused = fw[0] * m1 + fw[1] * m2 + fw[2] * m3 + fw[3] * m4
    hh = fused @ inp["fc1W"].T.astype(np.float32) + inp["fc1b"]
    hh = hh / (1 + np.exp(-hh))
    logits = hh @ inp["fc2W"].T.astype(np.float32) + inp["fc2b"]
    p = np.exp(logits - logits.max(1, keepdims=True))
    p /= p.sum(1, keepdims=True)
    out = (p @ inp["fc3W"].T.astype(np.float32) + inp["fc3b"]).reshape(B)
    return out.astype(np.float32)
